# revision 6
# baseline (speedup 1.0000x reference)
"""DGN (graph attention network) forward pass on 8 Trainium2 NeuronCores.

Strategy: pure data parallelism over the batch of 128 independent graphs
(16 graphs per core, weights replicated). Per graph, activations are kept
feature-major ([feature -> SBUF partitions, node -> free dim]) so weight
matrices serve directly as the stationary matmul operand. Attention is
computed k-major (scores^T[k, q]) so the softmax'd matrix feeds the AV
matmul without a transpose; softmax skips max-subtraction (scores are
tiny for this model family) and handles the mask via
  softmax(mask ? s : -inf) = mask*exp(s) / sum(mask*exp(s))
split as mask@V (head-shared) + ((exp(s)-1)*mask)@V for bf16 precision.
The denominator rides along as a ones-column appended to V.
"""

import os
import sys

for _p in ("/opt/trn_rl_repo",):
    if _p not in sys.path and os.path.isdir(_p):
        sys.path.append(_p)

import numpy as np

import concourse.bass as bass
import concourse.bacc as bacc
import concourse.tile as tile
from concourse import mybir
from concourse.masks import make_identity

F32 = mybir.dt.float32
BF16 = mybir.dt.bfloat16
I32 = mybir.dt.int32

B = 128          # total graphs
NCORES = 8
G = B // NCORES  # graphs per core
N = 256          # nodes per graph
NT = N // 128    # node tiles
F_IN = 128
HID = 512
KT = HID // 128  # K tiles over hidden dim
H = 8            # heads
D = 16           # head dim
HD = H * D       # 128
A = 32           # num actions
SCALE = 1.0 / (D ** 0.5)

WEIGHT_NAMES = [
    "enc_W1", "enc_b1", "enc_W2", "enc_b2",
    "Wv1", "bv1", "Wk1", "bk1", "Wq1", "bq1", "Wo1", "bo1",
    "Wv2", "bv2", "Wk2", "bk2", "Wq2", "bq2", "Wo2", "bo2",
    "q_W", "q_b",
]


def _emit(nc, tc, ap, g_count):
    """Emit the full per-core program. ap: dict name -> DRAM AP."""
    import contextlib
    ctx = contextlib.ExitStack()
    with ctx:
        # ---------------- pools ----------------
        wp = ctx.enter_context(tc.tile_pool(name="wp", bufs=1))       # persistent weights
        stg = ctx.enter_context(tc.tile_pool(name="stg", bufs=2))     # f32 weight staging
        gio = ctx.enter_context(tc.tile_pool(name="gio", bufs=3))     # per-graph dma-in tiles
        act = ctx.enter_context(tc.tile_pool(name="act", bufs=2))     # per-graph activations
        sml = ctx.enter_context(tc.tile_pool(name="sml", bufs=3))     # small per-use tiles
        esp = ctx.enter_context(tc.tile_pool(name="esp", bufs=4))     # exp / masked tiles
        pmm = ctx.enter_context(tc.tile_pool(name="pmm", bufs=2, space="PSUM"))  # [128,2,256] f32
        psc = ctx.enter_context(tc.tile_pool(name="psc", bufs=2, space="PSUM"))  # scores
        pav = ctx.enter_context(tc.tile_pool(name="pav", bufs=2, space="PSUM"))  # attention out
        ptr = ctx.enter_context(tc.tile_pool(name="ptr", bufs=2, space="PSUM"))  # transposes

        # ---------------- constants / weights ----------------
        eye = wp.tile([128, 128], BF16)
        make_identity(nc, eye)
        ones1 = wp.tile([1, 128], BF16)
        nc.vector.memset(ones1, 1.0)

        def load_cast(name, src_ap, shape):
            """DMA f32 DRAM -> staging -> bf16 weight tile."""
            st = stg.tile(shape, F32, tag="stage")
            nc.sync.dma_start(out=st, in_=src_ap)
            wt = wp.tile(shape, BF16, tag=name)
            nc.gpsimd.tensor_copy(out=wt, in_=st)
            return wt

        # encoder weights: lhsT layout [K(part), M]
        w1 = load_cast("w1", ap["enc_W1"], [128, HID])                       # [128, 512]
        w2 = load_cast("w2", ap["enc_W2"].rearrange("(k p) m -> p k m", p=128), [128, KT, HID])
        qw = load_cast("qw", ap["q_W"].rearrange("(k p) m -> p k m", p=128), [128, 3 * KT, A])

        # per-partition biases, feature-major: [128, n_mtiles]
        def load_bias_fm(name, n_mt):
            bt = wp.tile([128, n_mt], F32, tag="b_" + name)
            nc.sync.dma_start(out=bt, in_=ap[name].rearrange("(m p) -> p m", p=128))
            return bt

        b1 = load_bias_fm("enc_b1", KT)
        b2 = load_bias_fm("enc_b2", KT)

        qb = wp.tile([1, A], BF16)
        qb_st = stg.tile([1, A], F32, tag="stage_s")
        nc.sync.dma_start(out=qb_st, in_=ap["q_b"].rearrange("(o a) -> o a", o=1))
        nc.gpsimd.tensor_copy(out=qb, in_=qb_st)

        layers = []
        for li in (1, 2):
            wv = load_cast(f"wv{li}", ap[f"Wv{li}"].rearrange("(k p) m -> p k m", p=128), [128, KT, HD])
            wo = load_cast(f"wo{li}", ap[f"Wo{li}"], [128, HID])
            bo = load_bias_fm(f"bo{li}", KT)
            bv = wp.tile([128, 1], F32, tag=f"bv{li}")
            nc.sync.dma_start(out=bv, in_=ap[f"bv{li}"].rearrange("(p o) -> p o", o=1))

            # packed q/k weights: pack pk holds heads pk*4+i at column band 32*i..32*i+16
            packs = {}
            for nm in ("q", "k"):
                w_r = ap[f"W{nm}{li}"].rearrange("(k p) m -> p k m", p=128)
                for pk in range(2):
                    st = stg.tile([128, KT, 128], F32, tag="stage")
                    nc.vector.memset(st, 0.0)
                    for i in range(4):
                        hh = pk * 4 + i
                        nc.sync.dma_start(
                            out=st[:, :, 32 * i: 32 * i + D],
                            in_=w_r[:, :, D * hh: D * hh + D],
                        )
                    wt = wp.tile([128, KT, 128], BF16, tag=f"w{nm}{li}{pk}")
                    nc.gpsimd.tensor_copy(out=wt, in_=st)
                    bt = wp.tile([128, 1], F32, tag=f"b{nm}{li}{pk}")
                    nc.vector.memset(bt, 0.0)
                    for i in range(4):
                        hh = pk * 4 + i
                        nc.sync.dma_start(
                            out=bt[32 * i: 32 * i + D, 0:1],
                            in_=ap[f"b{nm}{li}"][D * hh: D * hh + D].rearrange("(p o) -> p o", o=1),
                        )
                    if nm == "q":
                        nc.scalar.mul(out=bt, in_=bt, mul=SCALE)
                    packs[(nm, pk)] = (wt, bt)
            layers.append(dict(wv=wv, bv=bv, wo=wo, bo=bo, packs=packs))

        # ---------------- per-graph program ----------------
        for g in range(g_count):
            # ---- loads ----
            x_st = gio.tile([128, NT, F_IN], F32, tag="x")
            nc.sync.dma_start(out=x_st, in_=ap["x"][g].rearrange("(t p) f -> p t f", p=128))
            m_i = gio.tile([128, NT, N], I32, tag="mi")
            nc.sync.dma_start(out=m_i, in_=ap["mask"][g].rearrange("(t p) k -> p t k", p=128))

            # ---- mask: cast + transpose -> maskT[k_in_tile, kt, q] (bf16) ----
            m_b = sml.tile([128, NT, N], BF16, tag="mb")
            nc.gpsimd.tensor_copy(out=m_b, in_=m_i)
            mT = sml.tile([128, NT, N], BF16, tag="mT")
            for kt in range(NT):
                ps = ptr.tile([128, NT, 128], BF16, tag="tr")
                for qt in range(NT):
                    nc.tensor.transpose(ps[:, qt, :], m_b[:, qt, 128 * kt: 128 * (kt + 1)], eye)
                nc.vector.tensor_copy(out=mT[:, kt, :].rearrange("p (t n) -> p t n", t=NT), in_=ps)

            # ---- x: cast + transpose -> xT [f, node] bf16 ----
            x_b = sml.tile([128, NT, F_IN], BF16, tag="xb")
            nc.scalar.copy(out=x_b, in_=x_st)
            ps = ptr.tile([128, NT, 128], BF16, tag="tr")
            for t in range(NT):
                nc.tensor.transpose(ps[:, t, :], x_b[:, t, :], eye)
            xT = sml.tile([128, N], BF16, tag="xT")
            nc.vector.tensor_copy(out=xT.rearrange("p (t n) -> p t n", t=NT), in_=ps)

            # ---- encoder ----
            h1 = sml.tile([128, KT, N], BF16, tag="h1")
            for half in range(2):
                ps = pmm.tile([128, 2, N], F32, tag="mm")
                for j in range(2):
                    mt = half * 2 + j
                    nc.tensor.matmul(ps[:, j, :], w1[:, 128 * mt: 128 * (mt + 1)], xT,
                                     start=True, stop=True)
                for j in range(2):
                    mt = half * 2 + j
                    nc.scalar.activation(out=h1[:, mt, :], in_=ps[:, j, :],
                                         func=mybir.ActivationFunctionType.Relu,
                                         bias=b1[:, mt: mt + 1], scale=1.0)
            h0 = act.tile([128, KT, N], BF16, tag="h0")
            for half in range(2):
                ps = pmm.tile([128, 2, N], F32, tag="mm")
                for j in range(2):
                    mt = half * 2 + j
                    for kt in range(KT):
                        nc.tensor.matmul(ps[:, j, :], w2[:, kt, 128 * mt: 128 * (mt + 1)],
                                         h1[:, kt, :], start=(kt == 0), stop=(kt == KT - 1))
                for j in range(2):
                    mt = half * 2 + j
                    nc.scalar.activation(out=h0[:, mt, :], in_=ps[:, j, :],
                                         func=mybir.ActivationFunctionType.Relu,
                                         bias=b2[:, mt: mt + 1], scale=1.0)

            # ---- attention layers ----
            h_in = h0
            h_keep = [h0]
            for li in range(2):
                L = layers[li]
                # q/k projections (packed), v projection (feature-major)
                qkt = {}
                for nm in ("q", "k"):
                    ps = pmm.tile([128, 2, N], F32, tag="mm")
                    for pk in range(2):
                        wt, bt = L["packs"][(nm, pk)]
                        for kt in range(KT):
                            nc.tensor.matmul(ps[:, pk, :], wt[:, kt, :], h_in[:, kt, :],
                                             start=(kt == 0), stop=(kt == KT - 1))
                    out_t = sml.tile([128, 2, N], BF16, tag=nm + "p")
                    for pk in range(2):
                        wt, bt = L["packs"][(nm, pk)]
                        nc.scalar.activation(out=out_t[:, pk, :], in_=ps[:, pk, :],
                                             func=mybir.ActivationFunctionType.Relu,
                                             bias=bt[:, 0:1],
                                             scale=SCALE if nm == "q" else 1.0)
                    qkt[nm] = out_t
                qp, kp = qkt["q"], qkt["k"]

                ps_v = pmm.tile([128, 2, N], F32, tag="mm")
                for kt in range(KT):
                    nc.tensor.matmul(ps_v[:, 0, :], L["wv"][:, kt, :], h_in[:, kt, :],
                                     start=(kt == 0), stop=(kt == KT - 1))
                vfm = sml.tile([128, N], BF16, tag="vfm")
                nc.vector.tensor_scalar(out=vfm, in0=ps_v[:, 0, :],
                                        scalar1=L["bv"][:, 0:1], scalar2=0.0,
                                        op0=mybir.AluOpType.add, op1=mybir.AluOpType.max)

                # v_ext[node_in_tile, t, 17h:17h+16] = v rows, col 17h+16 = 1.0
                v_ext = sml.tile([128, NT, 17 * H], BF16, tag="vext")
                ps = ptr.tile([128, NT, 128], BF16, tag="tr")
                for t in range(NT):
                    nc.tensor.transpose(ps[:, t, :], vfm[:, 128 * t: 128 * (t + 1)], eye)
                v_ext_r = v_ext.rearrange("p t (h c) -> p t h c", c=17)
                nc.vector.tensor_copy(out=v_ext_r[:, :, :, 0:D],
                                      in_=ps.rearrange("p t (h c) -> p t h c", c=D))
                nc.vector.memset(v_ext_r[:, :, :, D:17], 1.0)

                # scores + exp + masked delta, per head
                me_list = []
                for hh in range(H):
                    pk, band = hh // 4, 32 * (hh % 4)
                    ps_s = psc.tile([128, NT, N], F32, tag="sc")
                    for kt in range(NT):
                        nc.tensor.matmul(ps_s[:, kt, :],
                                         kp[band: band + D, pk, 128 * kt: 128 * (kt + 1)],
                                         qp[band: band + D, pk, :],
                                         start=(kt == 0), stop=(kt == NT - 1),
                                         tile_position=(band, 0))
                    e_s = esp.tile([128, NT, N], BF16, tag="es")
                    nc.scalar.activation(out=e_s, in_=ps_s,
                                         func=mybir.ActivationFunctionType.Exp)
                    me = esp.tile([128, NT, N], BF16, tag="me")
                    nc.vector.scalar_tensor_tensor(out=me, in0=e_s, scalar=-1.0,
                                                   in1=mT,
                                                   op0=mybir.AluOpType.add,
                                                   op1=mybir.AluOpType.mult)
                    me_list.append(me)

                # AV: base (mask @ v_ext, head-shared) + per-head delta.
                # One psum bank -> one accumulation group: start on the very
                # first matmul, stop on the very last.
                ps_o = pav.tile([128, NT, 17 * H], F32, tag="oext")
                first = True
                for qt in range(NT):
                    for kt in range(NT):
                        nc.tensor.matmul(ps_o[:, qt, :], mT[:, kt, 128 * qt: 128 * (qt + 1)],
                                         v_ext[:, kt, :], start=first, stop=False)
                        first = False
                for hh in range(H):
                    me = me_list[hh]
                    for qt in range(NT):
                        for kt in range(NT):
                            nc.tensor.matmul(ps_o[:, qt, 17 * hh: 17 * hh + 17],
                                             me[:, kt, 128 * qt: 128 * (qt + 1)],
                                             v_ext[:, kt, 17 * hh: 17 * hh + 17],
                                             start=False,
                                             stop=(hh == H - 1 and qt == NT - 1
                                                   and kt == NT - 1))

                # normalize + residual -> att (row-major), then transpose -> attT
                att = sml.tile([128, NT, HD], BF16, tag="att")
                ps_o_r = ps_o.rearrange("p t (h c) -> p t h c", c=17)
                for qt in range(NT):
                    rden = sml.tile([128, H], F32, tag="rden")
                    nc.vector.reciprocal(out=rden, in_=ps_o_r[:, qt, :, 16])
                    for hh in range(H):
                        nc.vector.scalar_tensor_tensor(
                            out=att[:, qt, D * hh: D * hh + D],
                            in0=ps_o_r[:, qt, hh, 0:D],
                            scalar=rden[:, hh: hh + 1],
                            in1=v_ext_r[:, qt, hh, 0:D],
                            op0=mybir.AluOpType.mult, op1=mybir.AluOpType.add)
                ps = ptr.tile([128, NT, 128], BF16, tag="tr")
                for qt in range(NT):
                    nc.tensor.transpose(ps[:, qt, :], att[:, qt, :], eye)
                attT = sml.tile([128, N], BF16, tag="attT")
                nc.vector.tensor_copy(out=attT.rearrange("p (t n) -> p t n", t=NT), in_=ps)

                # output projection
                h_out = act.tile([128, KT, N], BF16, tag=f"hL{li}")
                for half in range(2):
                    ps2 = pmm.tile([128, 2, N], F32, tag="mm")
                    for j in range(2):
                        mt = half * 2 + j
                        nc.tensor.matmul(ps2[:, j, :], L["wo"][:, 128 * mt: 128 * (mt + 1)],
                                         attT, start=True, stop=True)
                    for j in range(2):
                        mt = half * 2 + j
                        nc.scalar.activation(out=h_out[:, mt, :], in_=ps2[:, j, :],
                                             func=mybir.ActivationFunctionType.Relu,
                                             bias=L["bo"][:, mt: mt + 1], scale=1.0)
                h_keep.append(h_out)
                h_in = h_out

            # ---- final Q head: out[node, A] = concat(h_keep) @ q_W + q_b ----
            ps_f = ptr.tile([128, NT, A], F32, tag="tr")
            for qt in range(NT):
                nc.tensor.matmul(ps_f[:, qt, :], ones1, qb, start=True, stop=False)
                for j in range(3):
                    src = h_keep[j]
                    for kt in range(KT):
                        nc.tensor.matmul(ps_f[:, qt, :],
                                         src[:, kt, 128 * qt: 128 * (qt + 1)],
                                         qw[:, j * KT + kt, :],
                                         start=False,
                                         stop=(j == 2 and kt == KT - 1))
            o_sb = sml.tile([128, NT, A], F32, tag="osb")
            nc.scalar.copy(out=o_sb, in_=ps_f)
            nc.sync.dma_start(out=ap["out"][g].rearrange("(t p) a -> p t a", p=128), in_=o_sb)


def build(g_count=G, num_devices=NCORES):
    nc = bacc.Bacc("TRN2", target_bir_lowering=False, debug=False,
                   num_devices=num_devices)
    ap = {}
    ap["x"] = nc.dram_tensor("x", [g_count, N, F_IN], F32, kind="ExternalInput").ap()
    ap["mask"] = nc.dram_tensor("mask", [g_count, N, N], I32, kind="ExternalInput").ap()
    shapes = {
        "enc_W1": [F_IN, HID], "enc_b1": [HID], "enc_W2": [HID, HID], "enc_b2": [HID],
        "q_W": [3 * HID, A], "q_b": [A],
    }
    for li in (1, 2):
        shapes[f"Wv{li}"] = [HID, HD]; shapes[f"bv{li}"] = [HD]
        shapes[f"Wk{li}"] = [HID, HD]; shapes[f"bk{li}"] = [HD]
        shapes[f"Wq{li}"] = [HID, HD]; shapes[f"bq{li}"] = [HD]
        shapes[f"Wo{li}"] = [HD, HID]; shapes[f"bo{li}"] = [HID]
    for nm in WEIGHT_NAMES:
        ap[nm] = nc.dram_tensor(nm, shapes[nm], F32, kind="ExternalInput").ap()
    ap["out"] = nc.dram_tensor("out", [g_count, N, A], F32, kind="ExternalOutput").ap()

    with tile.TileContext(nc) as tc:
        _emit(nc, tc, ap, g_count)
    nc.compile()
    return nc


_NC_CACHE = {}


def kernel(**inputs):
    key = "full"
    if key not in _NC_CACHE:
        _NC_CACHE[key] = build(G, NCORES)
    nc = _NC_CACHE[key]

    from concourse import bass_utils
    in_maps = []
    for c in range(NCORES):
        m = {
            "x": np.ascontiguousarray(inputs["x"][c * G:(c + 1) * G], dtype=np.float32),
            "mask": np.ascontiguousarray(inputs["mask"][c * G:(c + 1) * G], dtype=np.int32),
        }
        for nm in WEIGHT_NAMES:
            m[nm] = np.ascontiguousarray(inputs[nm], dtype=np.float32)
        in_maps.append(m)
    res = bass_utils.run_bass_kernel_spmd(nc, in_maps, core_ids=list(range(NCORES)))
    return np.concatenate([r["out"] for r in res.results], axis=0)


# revision 44
# speedup vs baseline: 238.5139x; 238.5139x over previous
"""DGN (graph attention network) forward pass on 8 Trainium2 NeuronCores.

Strategy: pure data parallelism over the batch of 128 independent graphs
(16 graphs per core, weights replicated). Per graph, activations are kept
feature-major ([feature -> SBUF partitions, node -> free dim]) so weight
matrices serve directly as the stationary matmul operand. Attention is
computed k-major (scores^T[k, q]) so the softmax'd matrix feeds the AV
matmul without a transpose; softmax skips max-subtraction (scores are
tiny for this model family) and handles the mask via
  softmax(mask ? s : -inf) = mask*exp(s) / sum(mask*exp(s))
split as mask@V (head-shared) + ((exp(s)-1)*mask)@V for bf16 precision.
The denominator rides along as a ones-column appended to V.
"""

import os
import sys

for _p in ("/opt/trn_rl_repo",):
    if _p not in sys.path and os.path.isdir(_p):
        sys.path.append(_p)

import numpy as np

import concourse.bass as bass
import concourse.bacc as bacc
import concourse.tile as tile
from concourse import mybir
from concourse.masks import make_identity

F32 = mybir.dt.float32
BF16 = mybir.dt.bfloat16
I32 = mybir.dt.int32

B = 128          # total graphs
NCORES = 8
G = B // NCORES  # graphs per core
N = 256          # nodes per graph
NT = N // 128    # node tiles
F_IN = 128
HID = 512
KT = HID // 128  # K tiles over hidden dim
H = 8            # heads
D = 16           # head dim
HD = H * D       # 128
A = 32           # num actions
SCALE = 1.0 / (D ** 0.5)

WEIGHT_NAMES = [
    "enc_W1", "enc_b1", "enc_W2", "enc_b2",
    "Wv1", "bv1", "Wk1", "bk1", "Wq1", "bq1", "Wo1", "bo1",
    "Wv2", "bv2", "Wk2", "bk2", "Wq2", "bq2", "Wo2", "bo2",
    "q_W", "q_b",
]


def _emit(nc, tc, ap, g_count):
    """Emit the full per-core program. ap: dict name -> DRAM AP."""
    import contextlib
    ctx = contextlib.ExitStack()
    with ctx:
        # ---------------- pools ----------------
        wp = ctx.enter_context(tc.tile_pool(name="wp", bufs=1))       # persistent weights
        stg = ctx.enter_context(tc.tile_pool(name="stg", bufs=2))     # f32 weight staging
        gio = ctx.enter_context(tc.tile_pool(name="gio", bufs=4))     # per-graph dma-in tiles
        act = ctx.enter_context(tc.tile_pool(name="act", bufs=4))     # per-graph activations
        sml = ctx.enter_context(tc.tile_pool(name="sml", bufs=5))     # small per-use tiles
        esp = ctx.enter_context(tc.tile_pool(name="esp", bufs=6))     # exp tiles
        mep = ctx.enter_context(tc.tile_pool(name="mep", bufs=16))    # masked-exp tiles
        pmm = ctx.enter_context(tc.tile_pool(name="pmm", bufs=2, space="PSUM"))  # [128,2,256] f32
        psc = ctx.enter_context(tc.tile_pool(name="psc", bufs=2, space="PSUM"))  # scores
        pav = ctx.enter_context(tc.tile_pool(name="pav", bufs=2, space="PSUM"))  # attention out
        ptr = ctx.enter_context(tc.tile_pool(name="ptr", bufs=2, space="PSUM"))  # transposes

        # ---------------- constants / weights ----------------
        eye = wp.tile([128, 128], BF16)
        make_identity(nc, eye)
        ones1 = wp.tile([1, 128], BF16)
        nc.vector.memset(ones1, 1.0)
        # selector matrices for packing biases: sel_pk[16*(4*pk+i)+d, 32*i+d] = 1
        sels = []
        for pk in range(2):
            sel = wp.tile([128, 128], BF16, tag=f"sel{pk}")
            nc.vector.memset(sel.rearrange("p (b c) -> p b c", c=32)[:, :, D:32], 0.0)
            nc.vector.tensor_copy(
                out=sel.rearrange("p (b c) -> p b c", c=32)[:, :, 0:D],
                in_=eye[:, 64 * pk: 64 * pk + 64].rearrange("p (b c) -> p b c", c=D))
            sels.append(sel)

        _cast_engs = [nc.vector, nc.gpsimd, nc.scalar]
        _cast_i = [0]
        _dma_engs = [nc.sync]
        _dma_i = [0]

        def dma_rr(out, in_):
            eng = _dma_engs[_dma_i[0] % len(_dma_engs)]
            _dma_i[0] += 1
            eng.dma_start(out=out, in_=in_)

        def load_cast(name, src_ap, shape):
            """DMA f32 DRAM -> staging -> bf16 weight tile."""
            st = stg.tile(shape, F32, tag="stage")
            dma_rr(st, src_ap)
            wt = wp.tile(shape, BF16, tag=name)
            eng = _cast_engs[_cast_i[0] % 3]
            _cast_i[0] += 1
            if eng is nc.scalar:
                eng.copy(out=wt, in_=st)
            else:
                eng.tensor_copy(out=wt, in_=st)
            return wt

        # encoder weights: lhsT layout [K(part), M]
        w1 = load_cast("w1", ap["enc_W1"], [128, HID])                       # [128, 512]
        w2 = load_cast("w2", ap["enc_W2"].rearrange("(k p) m -> p k m", p=128), [128, KT, HID])
        qw = load_cast("qw", ap["q_W"].rearrange("(k p) m -> p k m", p=128), [128, 3 * KT, A])

        # per-partition biases, feature-major: [128, n_mtiles]
        def load_bias_fm(name, n_mt):
            bt = wp.tile([128, n_mt], F32, tag="b_" + name)
            dma_rr(bt, ap[name].rearrange("(m p) -> p m", p=128))
            return bt

        b1 = load_bias_fm("enc_b1", KT)
        b2 = load_bias_fm("enc_b2", KT)

        qb = wp.tile([1, A], BF16)
        qb_st = stg.tile([1, A], F32, tag="stage_s")
        dma_rr(qb_st, ap["q_b"].rearrange("(o a) -> o a", o=1))
        nc.gpsimd.tensor_copy(out=qb, in_=qb_st)

        layers = []
        for li in (1, 2):
            wv = load_cast(f"wv{li}", ap[f"Wv{li}"].rearrange("(k p) m -> p k m", p=128), [128, KT, HD])
            wo = load_cast(f"wo{li}", ap[f"Wo{li}"], [128, HID])
            bo = load_bias_fm(f"bo{li}", KT)
            bv = wp.tile([128, 1], F32, tag=f"bv{li}")
            dma_rr(bv, ap[f"bv{li}"].rearrange("(p o) -> p o", o=1))

            # packed q/k weights: pack pk holds heads pk*4+i at column band
            # 32*i..32*i+16. One natural-layout DMA per tensor; the packing is
            # a strided on-chip copy (cast included). Gap columns never feed
            # a matmul slice, so they are left unzeroed.
            packs = {}
            bnat = {}
            for nm in ("q", "k"):
                bn = stg.tile([128, 1], BF16, tag="bnat_" + nm)
                bn_f = stg.tile([128, 1], F32, tag="bnatf_" + nm)
                nc.sync.dma_start(out=bn_f, in_=ap[f"b{nm}{li}"].rearrange("(p o) -> p o", o=1))
                nc.vector.tensor_copy(out=bn, in_=bn_f)
                bnat[nm] = bn
            for nm in ("q", "k"):
                w_r = ap[f"W{nm}{li}"].rearrange("(k p) m -> p k m", p=128)
                stn = stg.tile([128, KT, 128], F32, tag="stage")
                nc.sync.dma_start(out=stn, in_=w_r)
                for pk in range(2):
                    wt = wp.tile([128, KT, 128], BF16, tag=f"w{nm}{li}{pk}")
                    nc.vector.memset(wt.rearrange("p k (b c) -> p k b c", c=32)[:, :, :, D:32], 0.0)
                    eng = _cast_engs[_cast_i[0] % 3]
                    _cast_i[0] += 1
                    dst = wt.rearrange("p k (b c) -> p k b c", c=32)[:, :, :, 0:D]
                    srcv = stn[:, :, 64 * pk: 64 * pk + 64].rearrange(
                        "p k (b c) -> p k b c", c=D)
                    if eng is nc.scalar:
                        eng.copy(out=dst, in_=srcv)
                    else:
                        eng.tensor_copy(out=dst, in_=srcv)
                    bt = wp.tile([128, 1], F32, tag=f"b{nm}{li}{pk}")
                    ps_b = ptr.tile([128, NT, 64], F32, tag="tr")
                    nc.tensor.matmul(ps_b[:, 0, 0:1], sels[pk], bnat[nm],
                                     start=True, stop=True)
                    nc.vector.tensor_copy(out=bt, in_=ps_b[:, 0, 0:1])
                    if nm == "q":
                        nc.scalar.mul(out=bt, in_=bt, mul=SCALE)
                    packs[(nm, pk)] = (wt, bt)
            layers.append(dict(wv=wv, bv=bv, wo=wo, bo=bo, packs=packs))

        # ---------------- per-pair program ----------------
        # Graphs are processed in PAIRS: every weight-stationary matmul
        # (encoder, q/k/v projections, output projection) uses a moving
        # operand that spans both graphs' nodes (N=512), so each LDWEIGHTS
        # is amortized over two graphs and instruction counts halve.
        # Attention itself (scores, exp, AV) stays per-graph.
        # Emitted as generators with yields at phase boundaries so pairs
        # interleave in each engine's FIFO (queues run in emission order).
        def pair_prog(gs):
            W = N * len(gs)          # moving-operand width for shared matmuls

            # ---- per-graph loads + mask/x prep ----
            mT_l, xq = [], []
            for g in gs:
                x_st = gio.tile([128, NT, F_IN], F32, tag="x")
                nc.sync.dma_start(out=x_st, in_=ap["x"][g].rearrange("(t p) f -> p t f", p=128))
                m_i = gio.tile([128, NT, N], I32, tag="mi")
                nc.sync.dma_start(out=m_i, in_=ap["mask"][g].rearrange("(t p) k -> p t k", p=128))
                m_b = sml.tile([128, NT, N], BF16, tag="mb")
                nc.gpsimd.tensor_copy(out=m_b, in_=m_i)
                mT = sml.tile([128, NT, N], BF16, tag="mT")
                for kt in range(NT):
                    ps = ptr.tile([128, NT, 128], BF16, tag="tr")
                    for qt in range(NT):
                        nc.tensor.transpose(ps[:, qt, :], m_b[:, qt, 128 * kt: 128 * (kt + 1)], eye)
                    nc.vector.tensor_copy(out=mT[:, kt, :].rearrange("p (t n) -> p t n", t=NT), in_=ps)
                mT_l.append(mT)
                xq.append((x_st, m_b))
            yield

            xT = sml.tile([128, len(gs), N], BF16, tag="xT")
            for gi, g in enumerate(gs):
                x_st, _ = xq[gi]
                x_b = sml.tile([128, NT, F_IN], BF16, tag="xb")
                nc.gpsimd.tensor_copy(out=x_b, in_=x_st)
                ps = ptr.tile([128, NT, 128], BF16, tag="tr")
                for t in range(NT):
                    nc.tensor.transpose(ps[:, t, :], x_b[:, t, :], eye)
                nc.vector.tensor_copy(out=xT[:, gi, :].rearrange("p (t n) -> p t n", t=NT), in_=ps)
            yield

            # ---- encoder (pair-wide N=W matmuls) ----
            h1 = sml.tile([128, KT, len(gs), N], BF16, tag="h1")
            for half in range(2):
                for j in range(2):
                    mt = half * 2 + j
                    ps = pmm.tile([128, len(gs), N], F32, tag="mm")
                    nc.tensor.matmul(ps.rearrange("p g n -> p (g n)"),
                                     w1[:, 128 * mt: 128 * (mt + 1)],
                                     xT.rearrange("p g n -> p (g n)"),
                                     start=True, stop=True)
                    nc.scalar.activation(out=h1[:, mt, :, :], in_=ps,
                                         func=mybir.ActivationFunctionType.Relu,
                                         bias=b1[:, mt: mt + 1], scale=1.0)
                yield
            h0 = act.tile([128, KT, len(gs), N], BF16, tag="h0")
            for half in range(2):
                for j in range(2):
                    mt = half * 2 + j
                    ps = pmm.tile([128, len(gs), N], F32, tag="mm")
                    for kt in range(KT):
                        nc.tensor.matmul(ps.rearrange("p g n -> p (g n)"),
                                         w2[:, kt, 128 * mt: 128 * (mt + 1)],
                                         h1[:, kt, :, :].rearrange("p g n -> p (g n)"),
                                         start=(kt == 0), stop=(kt == KT - 1))
                    nc.scalar.activation(out=h0[:, mt, :, :], in_=ps,
                                         func=mybir.ActivationFunctionType.Relu,
                                         bias=b2[:, mt: mt + 1], scale=1.0)
                yield

            # ---- attention layers ----
            h_in = h0
            h_keep = [h0]
            for li in range(2):
                L = layers[li]
                # q/k projections (packed, pair-wide)
                qkt = {}
                for nm in ("q", "k"):
                    out_t = sml.tile([128, 2, len(gs), N], BF16, tag=nm + "p")
                    for pk in range(2):
                        wt, bt = L["packs"][(nm, pk)]
                        ps = pmm.tile([128, len(gs), N], F32, tag="mm")
                        for kt in range(KT):
                            nc.tensor.matmul(ps.rearrange("p g n -> p (g n)"),
                                             wt[:, kt, :],
                                             h_in[:, kt, :, :].rearrange("p g n -> p (g n)"),
                                             start=(kt == 0), stop=(kt == KT - 1))
                        nc.scalar.activation(out=out_t[:, pk, :, :], in_=ps,
                                             func=mybir.ActivationFunctionType.Relu,
                                             bias=bt[:, 0:1],
                                             scale=SCALE if nm == "q" else 1.0)
                    qkt[nm] = out_t
                    yield
                qp, kp = qkt["q"], qkt["k"]

                # v projection (pair-wide), then per-graph v_ext
                ps_v = pmm.tile([128, len(gs), N], F32, tag="mm")
                for kt in range(KT):
                    nc.tensor.matmul(ps_v.rearrange("p g n -> p (g n)"),
                                     L["wv"][:, kt, :],
                                     h_in[:, kt, :, :].rearrange("p g n -> p (g n)"),
                                     start=(kt == 0), stop=(kt == KT - 1))
                vfm = sml.tile([128, len(gs), N], BF16, tag="vfm")
                nc.vector.tensor_scalar(out=vfm, in0=ps_v,
                                        scalar1=L["bv"][:, 0:1], scalar2=0.0,
                                        op0=mybir.AluOpType.add, op1=mybir.AluOpType.max)
                v_ext_l, v_ext_r_l = [], []
                for gi in range(len(gs)):
                    v_ext = sml.tile([128, NT, 17 * H], BF16, tag="vext")
                    ps = ptr.tile([128, NT, 128], BF16, tag="tr")
                    for t in range(NT):
                        nc.tensor.transpose(ps[:, t, :], vfm[:, gi, 128 * t: 128 * (t + 1)], eye)
                    v_ext_r = v_ext.rearrange("p t (h c) -> p t h c", c=17)
                    nc.vector.tensor_copy(out=v_ext_r[:, :, :, 0:D],
                                          in_=ps.rearrange("p t (h c) -> p t h c", c=D))
                    nc.vector.memset(v_ext_r[:, :, :, D:17], 1.0)
                    v_ext_l.append(v_ext)
                    v_ext_r_l.append(v_ext_r)
                yield

                # scores + exp + masked delta, per graph, heads in pairs.
                # Consecutive matmuls alternate 32-row bands (distinct PE row
                # groups + distinct psum banks) so weight loads can overlap
                # the previous matmul.
                me_l = [[] for _ in gs]
                for hp in range(H // 2):
                    h0x, h1x = 2 * hp, 2 * hp + 1
                    for gi in range(len(gs)):
                        ps_sa = psc.tile([128, NT, N], F32, tag="sc")
                        ps_sb = psc.tile([128, NT, N], F32, tag="sc")
                        pss = {h0x: ps_sa, h1x: ps_sb}
                        for kt in range(NT):
                            for hh in (h0x, h1x):
                                pk, band = hh // 4, 32 * (hh % 4)
                                nc.tensor.matmul(pss[hh][:, kt, :],
                                                 kp[band: band + D, pk, gi, 128 * kt: 128 * (kt + 1)],
                                                 qp[band: band + D, pk, gi, :],
                                                 start=(kt == 0), stop=(kt == NT - 1),
                                                 tile_position=(band, 0))
                        for hh in (h0x, h1x):
                            e_s = esp.tile([128, NT, N], BF16, tag="es")
                            nc.scalar.activation(out=e_s, in_=pss[hh],
                                                 func=mybir.ActivationFunctionType.Exp)
                            me = mep.tile([128, NT, N], BF16, tag="me")
                            nc.vector.scalar_tensor_tensor(out=me, in0=e_s, scalar=-1.0,
                                                           in1=mT_l[gi],
                                                           op0=mybir.AluOpType.add,
                                                           op1=mybir.AluOpType.mult)
                            me_l[gi].append(me)
                    yield

                # AV per graph: base + per-head deltas; one accumulation
                # group per psum bank (start on first, stop on last).
                ps_o_l = []
                for gi in range(len(gs)):
                    mT = mT_l[gi]
                    v_ext = v_ext_l[gi]
                    ps_o = pav.tile([128, NT, 17 * H], F32, tag="oext")
                    first = True
                    for qt in range(NT):
                        for kt in range(NT):
                            nc.tensor.matmul(ps_o[:, qt, :], mT[:, kt, 128 * qt: 128 * (qt + 1)],
                                             v_ext[:, kt, :], start=first, stop=False)
                            first = False
                    for hh in range(H):
                        me = me_l[gi][hh]
                        for qt in range(NT):
                            for kt in range(NT):
                                nc.tensor.matmul(ps_o[:, qt, 17 * hh: 17 * hh + 17],
                                                 me[:, kt, 128 * qt: 128 * (qt + 1)],
                                                 v_ext[:, kt, 17 * hh: 17 * hh + 17],
                                                 start=False,
                                                 stop=(hh == H - 1 and qt == NT - 1
                                                       and kt == NT - 1))
                    ps_o_l.append(ps_o)
                    yield

                # normalize + residual + transpose -> attT (both graphs)
                attT = sml.tile([128, len(gs), N], BF16, tag="attT")
                for gi in range(len(gs)):
                    ps_o_r = ps_o_l[gi].rearrange("p t (h c) -> p t h c", c=17)
                    att = sml.tile([128, NT, HD], BF16, tag="att")
                    for qt in range(NT):
                        rden = sml.tile([128, H], F32, tag="rden")
                        nc.vector.reciprocal(out=rden, in_=ps_o_r[:, qt, :, 16])
                        den_b = sml.tile([128, H, D], BF16, tag="denb")
                        rden_bc = bass.AP(tensor=rden.tensor, offset=rden.offset,
                                          ap=[rden.ap[0], rden.ap[1], [0, D]])
                        nc.vector.tensor_copy(out=den_b, in_=rden_bc)
                        att_r = att[:, qt, :].rearrange("p (h c) -> p h c", c=D)
                        nc.vector.tensor_mul(out=att_r, in0=ps_o_r[:, qt, :, 0:D],
                                             in1=den_b)
                        nc.vector.tensor_add(out=att_r, in0=att_r,
                                             in1=v_ext_r_l[gi][:, qt, :, 0:D])
                    ps = ptr.tile([128, NT, 128], BF16, tag="tr")
                    for qt in range(NT):
                        nc.tensor.transpose(ps[:, qt, :], att[:, qt, :], eye)
                    nc.vector.tensor_copy(out=attT[:, gi, :].rearrange("p (t n) -> p t n", t=NT), in_=ps)
                    yield

                # output projection (pair-wide)
                h_out = act.tile([128, KT, len(gs), N], BF16, tag=f"hL{li}")
                for half in range(2):
                    for j in range(2):
                        mt = half * 2 + j
                        ps2 = pmm.tile([128, len(gs), N], F32, tag="mm")
                        nc.tensor.matmul(ps2.rearrange("p g n -> p (g n)"),
                                         L["wo"][:, 128 * mt: 128 * (mt + 1)],
                                         attT.rearrange("p g n -> p (g n)"),
                                         start=True, stop=True)
                        nc.scalar.activation(out=h_out[:, mt, :, :], in_=ps2,
                                             func=mybir.ActivationFunctionType.Relu,
                                             bias=L["bo"][:, mt: mt + 1], scale=1.0)
                    yield
                h_keep.append(h_out)
                h_in = h_out

            # ---- final Q head (per graph; LDWEIGHTS here is tiny) ----
            for gi, g in enumerate(gs):
                ps_f = ptr.tile([128, NT, A], F32, tag="tr")
                for qt in range(NT):
                    nc.tensor.matmul(ps_f[:, qt, :], ones1, qb, start=True, stop=False)
                    for j in range(3):
                        src_t = h_keep[j]
                        for kt in range(KT):
                            nc.tensor.matmul(ps_f[:, qt, :],
                                             src_t[:, kt, gi, 128 * qt: 128 * (qt + 1)],
                                             qw[:, j * KT + kt, :],
                                             start=False,
                                             stop=(j == 2 and kt == KT - 1))
                o_sb = sml.tile([128, NT, A], F32, tag="osb")
                nc.vector.tensor_copy(out=o_sb, in_=ps_f)
                nc.sync.dma_start(out=ap["out"][g].rearrange("(t p) a -> p t a", p=128), in_=o_sb)
                yield

        # Drive the pair generators PIPE at a time, round-robin by phase,
        # with staggered starts so active pairs sit in different phases.
        PIPE = 2
        STAGGER = 7
        pairs = [list(range(i, min(i + 2, g_count))) for i in range(0, g_count, 2)]
        active = [pair_prog(pairs.pop(0))]
        rounds = 0
        while pairs or active:
            rounds += 1
            if rounds % STAGGER == 0 and len(active) < PIPE and pairs:
                active.append(pair_prog(pairs.pop(0)))
            for gen in list(active):
                try:
                    next(gen)
                except StopIteration:
                    active.remove(gen)
                    if pairs:
                        active.append(pair_prog(pairs.pop(0)))


def build(g_count=G, num_devices=NCORES):
    nc = bacc.Bacc("TRN2", target_bir_lowering=False, debug=False,
                   num_devices=num_devices)
    ap = {}
    ap["x"] = nc.dram_tensor("x", [g_count, N, F_IN], F32, kind="ExternalInput").ap()
    ap["mask"] = nc.dram_tensor("mask", [g_count, N, N], I32, kind="ExternalInput").ap()
    shapes = {
        "enc_W1": [F_IN, HID], "enc_b1": [HID], "enc_W2": [HID, HID], "enc_b2": [HID],
        "q_W": [3 * HID, A], "q_b": [A],
    }
    for li in (1, 2):
        shapes[f"Wv{li}"] = [HID, HD]; shapes[f"bv{li}"] = [HD]
        shapes[f"Wk{li}"] = [HID, HD]; shapes[f"bk{li}"] = [HD]
        shapes[f"Wq{li}"] = [HID, HD]; shapes[f"bq{li}"] = [HD]
        shapes[f"Wo{li}"] = [HD, HID]; shapes[f"bo{li}"] = [HID]
    for nm in WEIGHT_NAMES:
        ap[nm] = nc.dram_tensor(nm, shapes[nm], F32, kind="ExternalInput").ap()
    ap["out"] = nc.dram_tensor("out", [g_count, N, A], F32, kind="ExternalOutput").ap()

    with tile.TileContext(nc) as tc:
        _emit(nc, tc, ap, g_count)
    nc.compile()
    return nc


_NC_CACHE = {}


def kernel(**inputs):
    key = "full"
    if key not in _NC_CACHE:
        _NC_CACHE[key] = build(G, NCORES)
    nc = _NC_CACHE[key]

    from concourse import bass_utils
    in_maps = []
    for c in range(NCORES):
        m = {
            "x": np.ascontiguousarray(inputs["x"][c * G:(c + 1) * G], dtype=np.float32),
            "mask": np.ascontiguousarray(inputs["mask"][c * G:(c + 1) * G], dtype=np.int32),
        }
        for nm in WEIGHT_NAMES:
            m[nm] = np.ascontiguousarray(inputs[nm], dtype=np.float32)
        in_maps.append(m)
    res = bass_utils.run_bass_kernel_spmd(nc, in_maps, core_ids=list(range(NCORES)))
    return np.concatenate([r["out"] for r in res.results], axis=0)
